# revision 8
# baseline (speedup 1.0000x reference)
"""KMeans (vq_codebook) Trainium2 Bass kernel.

Strategy (hardcoded for inputs: points [2,65536,3] f32, features [2,128,65536] f32,
centroids [2,512,3] f32):
  - 8 cores: 4 cores per batch, each owns a 16384-point shard of that batch.
  - Device: 15 Lloyd iterations. Per point-tile [128 pts]:
      sim = 2 p.c - |p|^2 - |c|^2 via one K=5 matmul into PSUM,
      rowmax (DVE reduce), onehot = (sim >= rowmax) (DVE), and a scatter
      matmul pts_aug^T @ onehot accumulating per-cluster sums+counts [4,512].
    Per-cluster sums+counts are AllReduced across the 4-core group each
    iteration; centroid update happens redundantly on every core.
    Final iteration also emits labels; a 16th sweep vs the updated centroids
    emits s_i = sim(p_i, c_final[label_i]).
  - Host: per-cluster top-15 of s (exact, tie-broken like jax.lax.top_k),
    softmax, cf = features[:, idx] @ w. (512*15 gathers - negligible.)
"""

import sys

sys.path.insert(0, "/opt/trn_rl_repo")

import numpy as np

import concourse.bass as bass
import concourse.bass_isa as bass_isa
import concourse.mybir as mybir
from concourse import bacc
from concourse import tile as tile_mod
from concourse.bass_utils import run_bass_kernel_spmd

F32 = mybir.dt.float32
AOT = mybir.AluOpType

B = 2
N = 65536
NF = 128
M = 512
D = 3
MAX_ITER = 15
CORES = 8
GROUP = 4          # cores per batch
NSH = N // GROUP   # 16384 points per core
NT = NSH // 128    # 128 point tiles per core
USE_F32R = False   # float32r is TF32-like reduced precision - not usable here


def _mm_dt(ap):
    return ap.bitcast(mybir.dt.float32r) if USE_F32R else ap


def build_nc():
    nc = bacc.Bacc(None, target_bir_lowering=False, num_devices=CORES)

    # ---- I/O ----
    ptsT_in = nc.dram_tensor("ptsT", [5, NSH], F32, kind="ExternalInput")
    ptsA_in = nc.dram_tensor("ptsA", [128, NT * 4], F32, kind="ExternalInput")
    iota_in = nc.dram_tensor("iota512", [128, M], F32, kind="ExternalInput")
    rhs0_in = nc.dram_tensor("rhs0", [5, M], F32, kind="ExternalInput")
    cen_out = nc.dram_tensor("cen_out", [3, M], F32, kind="ExternalOutput")
    lab_out = nc.dram_tensor("lab_out", [128, NT], F32, kind="ExternalOutput")
    s_out = nc.dram_tensor("s_out", [128, NT], F32, kind="ExternalOutput")

    with tile_mod.TileContext(nc) as tc:
        with (
            tc.tile_pool(name="persist", bufs=1) as pp,
            tc.tile_pool(name="onehot", bufs=4) as ohp,
            tc.tile_pool(name="rmax", bufs=8) as rmp,
            tc.tile_pool(name="psim", bufs=6, space="PSUM") as psp,
            tc.tile_pool(name="psums", bufs=2, space="PSUM") as pap,
            tc.tile_pool(name="dram", bufs=4, space="DRAM") as dp,
        ):
            ptsT = pp.tile([5, NSH], F32, tag="ptsT")
            ptsA = pp.tile([128, NT * 4], F32, tag="ptsA")
            iota = pp.tile([128, M], F32, tag="iota")
            rhs_c = pp.tile([5, M], F32, tag="rhs_c")
            lab_all = pp.tile([128, NT], F32, tag="lab")
            s_all = pp.tile([128, NT], F32, tag="s")
            sums_sb = pp.tile([4, M], F32, tag="sums_sb")
            sums_g = pp.tile([4, M], F32, tag="sums_g")
            scr_a = pp.tile([1, M], F32, tag="scr_a")
            scr_b = pp.tile([1, M], F32, tag="scr_b")

            nc.sync.dma_start(ptsT[:], ptsT_in[:])
            nc.sync.dma_start(ptsA[:], ptsA_in[:])
            nc.sync.dma_start(iota[:], iota_in[:])
            nc.sync.dma_start(rhs_c[:], rhs0_in[:])

            for it in range(MAX_ITER):
                last = it == MAX_ITER - 1
                sums = pap.tile([4, M], F32, tag="acc")
                for t in range(NT):
                    sim = psp.tile([128, M], F32, tag="sim")
                    nc.tensor.matmul(
                        sim[:],
                        _mm_dt(ptsT[:, t * 128:(t + 1) * 128]),
                        _mm_dt(rhs_c[:]),
                    )
                    rmax = rmp.tile([128, 1], F32, tag="rmax")
                    nc.vector.reduce_max(rmax[:], sim[:], axis=mybir.AxisListType.X)
                    oh = ohp.tile([128, M], F32, tag="oh")
                    nc.vector.tensor_scalar(
                        oh[:], sim[:], rmax[:], None, AOT.is_ge
                    )
                    if last:
                        # labels = sum_c onehot * iota  (argmax index)
                        scrL = ohp.tile([128, M], F32, tag="scrL")
                        nc.vector.scalar_tensor_tensor(
                            scrL[:], oh[:], 1.0, iota[:],
                            AOT.mult, AOT.mult,
                            accum_out=lab_all[:, t:t + 1],
                        )
                    nc.tensor.matmul(
                        sums[:],
                        _mm_dt(ptsA[:, t * 4:(t + 1) * 4]),
                        _mm_dt(oh[:]),
                        start=(t == 0),
                        stop=(t == NT - 1),
                    )

                # move partial sums to SBUF, allreduce over the 4-core group
                nc.vector.tensor_copy(sums_sb[:], sums[:])
                cc_in = dp.tile([4, M], F32, tag="cc_in")
                cc_out = dp.tile([4, M], F32, tag="cc_out")
                nc.sync.dma_start(cc_in[:], sums_sb[:])
                nc.gpsimd.collective_compute(
                    "AllReduce",
                    AOT.add,
                    replica_groups=[[0, 1, 2, 3], [4, 5, 6, 7]],
                    ins=[cc_in.opt()],
                    outs=[cc_out.opt()],
                )
                nc.sync.dma_start(sums_g[:], cc_out[:])

                # centroid update: c = sums / (cnt + 1e-8)
                # (engine ops must start at partition 0 -> move cnt row via DMA)
                nc.sync.dma_start(scr_a[:], sums_g[3:4, :])
                nc.vector.tensor_scalar_add(scr_a[:], scr_a[:], 1e-8)
                nc.vector.reciprocal(scr_b[:], scr_a[:])
                rec3 = rmp.tile([3, M], F32, tag="rec3")
                nc.gpsimd.partition_broadcast(rec3[:], scr_b[:], channels=3)
                nc.vector.tensor_tensor(
                    rhs_c[0:3, :], sums_g[0:3, :], rec3[:], AOT.mult
                )
                # rhs row 3 = -|c|^2  (square rows 0-2, cross-partition sum)
                sq3 = rmp.tile([3, M], F32, tag="sq3")
                nc.vector.tensor_tensor(
                    sq3[:], rhs_c[0:3, :], rhs_c[0:3, :], AOT.mult
                )
                nc.gpsimd.partition_all_reduce(
                    rec3[:], sq3[:], channels=3,
                    reduce_op=bass_isa.ReduceOp.add,
                )
                nc.vector.tensor_scalar_mul(scr_a[:], rec3[0:1, :], -1.0)
                nc.sync.dma_start(rhs_c[3:4, :], scr_a[:])
                # row 4 stays -1

            # ---- phase 2a: s_i = sim(p_i, c_final[label_i]) ----
            for t in range(NT):
                sim = psp.tile([128, M], F32, tag="sim")
                nc.tensor.matmul(
                    sim[:],
                    _mm_dt(ptsT[:, t * 128:(t + 1) * 128]),
                    _mm_dt(rhs_c[:]),
                )
                scr2 = ohp.tile([128, M], F32, tag="oh")
                nc.vector.scalar_tensor_tensor(
                    scr2[:], iota[:], lab_all[:, t:t + 1], sim[:],
                    AOT.is_equal, AOT.mult,
                    accum_out=s_all[:, t:t + 1],
                )

            nc.sync.dma_start(cen_out[:], rhs_c[0:3, :])
            nc.sync.dma_start(lab_out[:], lab_all[:])
            nc.sync.dma_start(s_out[:], s_all[:])

    nc.finalize()
    return nc


def _prep_core_inputs(points_b, cen_b, shard):
    p = points_b[shard * NSH:(shard + 1) * NSH]          # [16384, 3]
    p2 = np.sum(p * p, axis=1)
    ptsT = np.concatenate(
        [2.0 * p.T, np.ones((1, NSH), np.float32), p2[None, :]], axis=0
    ).astype(np.float32)                                  # [5, 16384]
    aug = np.concatenate([p, np.ones((NSH, 1), np.float32)], axis=1)  # [16384,4]
    ptsA = (
        aug.reshape(NT, 128, 4).transpose(1, 0, 2).reshape(128, NT * 4)
    ).astype(np.float32)
    c = cen_b                                             # [512, 3]
    c2 = np.sum(c * c, axis=1)
    rhs0 = np.concatenate(
        [c.T, -c2[None, :], -np.ones((1, M), np.float32)], axis=0
    ).astype(np.float32)                                  # [5, 512]
    iota512 = np.broadcast_to(
        np.arange(M, dtype=np.float32)[None, :], (128, M)
    ).copy()
    return {
        "ptsT": np.ascontiguousarray(ptsT),
        "ptsA": np.ascontiguousarray(ptsA),
        "iota512": iota512,
        "rhs0": np.ascontiguousarray(rhs0),
    }


_NC_CACHE = {}


def kernel(points, features, centroids):
    points = np.asarray(points, np.float32)
    features = np.asarray(features, np.float32)
    centroids = np.asarray(centroids, np.float32)

    if "nc" not in _NC_CACHE:
        _NC_CACHE["nc"] = build_nc()
    nc = _NC_CACHE["nc"]

    in_maps = []
    for core in range(CORES):
        b, shard = core // GROUP, core % GROUP
        in_maps.append(_prep_core_inputs(points[b], centroids[b], shard))

    try:
        res = run_bass_kernel_spmd(
            nc, in_maps, core_ids=list(range(CORES)),
            trace=bool(_NC_CACHE.get("trace")),
        )
    except Exception:
        if not _NC_CACHE.get("trace"):
            raise
        _NC_CACHE["trace"] = False
        res = run_bass_kernel_spmd(nc, in_maps, core_ids=list(range(CORES)))
    _NC_CACHE["last_result"] = res
    outs = res.results

    cc = np.zeros((B, M, D), np.float32)
    cf = np.zeros((B, NF, M), np.float32)
    lbl = np.zeros((B, N), np.int32)
    for b in range(B):
        cc[b] = outs[b * GROUP]["cen_out"].T
        labs = np.concatenate(
            [outs[b * GROUP + q]["lab_out"].T.ravel() for q in range(GROUP)]
        )
        svals = np.concatenate(
            [outs[b * GROUP + q]["s_out"].T.ravel() for q in range(GROUP)]
        )
        lbl[b] = labs.astype(np.int32)
        # host: per-cluster top-15 (ties -> lowest index, like lax.top_k),
        # softmax over the 15 scores, weighted feature gather.
        order = np.lexsort((np.arange(N), -svals.astype(np.float64)))
        ls = lbl[b][order]
        grp_sorted = np.lexsort((np.arange(N), ls))  # stable by label
        order = order[grp_sorted]
        ls = lbl[b][order]
        starts = np.searchsorted(ls, np.arange(M), side="left")
        counts = np.searchsorted(ls, np.arange(M), side="right") - starts
        if counts.min() >= 15:
            idx15 = np.empty((M, 15), np.int64)
            for k in range(15):
                idx15[:, k] = order[starts + k]
            s15 = svals[idx15]
        else:
            # rare: cluster with <15 members - replicate lax.top_k on the
            # masked [m, n] row (NEG_INF for non-members, ties -> low index)
            idx15 = np.empty((M, 15), np.int64)
            s15 = np.full((M, 15), -1e30, np.float32)
            for m in range(M):
                c = int(counts[m])
                mem = order[starts[m]:starts[m] + min(c, 15)]
                idx15[m, :len(mem)] = mem
                s15[m, :len(mem)] = svals[mem]
                if c < 15:
                    pad = [i for i in range(15 - c)]
                    idx15[m, c:] = pad
        s15 = s15.astype(np.float32)
        w = np.exp(s15 - s15.max(axis=1, keepdims=True))
        w /= w.sum(axis=1, keepdims=True)
        cf[b] = np.einsum("fmk,mk->fm", features[b][:, idx15], w.astype(np.float32))
    return cc, cf, lbl
